# revision 18
# baseline (speedup 1.0000x reference)
"""AtomTransformer sparse-window attention on 8 TRN2 NeuronCores.

Sharding (per the C-axis hint): the 64 windows are split 8-per-core.
The host performs the shard-defining index gathers (query_idx /
key_idx / pair_idx row gathers out of ql / cl / plm) to build each
core's private window blocks, every core runs an identical static
dense per-window attention program (LN + QKV projections + pair-bias
+ softmax + PV + output projection), and the host unshard step is the
scatter-add of the per-core query tiles plus the residual.  No
cross-device communication is needed.

Device-side layout notes (one window, all 8 heads fused; score
columns are laid out h*32+q):
  - lnkT (d1, k) / lnqT (d1, q) via PE transpose of the LN'd rows.
  - kwT (h*dh, k) = Wk^T @ lnkT; vw (k, h*dh) = lnk @ Wv;
    qwT (h*dh, q) = (Wq/4)^T @ lnqT  (score scale folded into Wq).
  - qbd (h*dh, h'*32+q) = block-diag placement of qwT, built in ONE
    DVE multiply against a constant 0/1 mask with a step-0 free-dim
    broadcast of qwT (per-head 16-row SBUF slices and partition-offset
    matmul operands are rejected by this toolchain, so all matmuls use
    full-K operands with masked/zero rows instead).
  - S (k, 256) accumulates in PSUM: 4 pair-bias matmuls (lhsT = pw
    chunk with partition (q%8, d), rhs = host-expanded block-diagonal
    Wb) + 1 score matmul (lhsT=kwT, rhs=qbd).
  - softmax over k (= partitions): exp on ACT; masked-key rows zeroed
    by a per-partition scalar multiply; column sums via a ones-matrix
    matmul (every partition receives the sum, so no cross-partition
    broadcast); reciprocal + multiply on DVE.
  - O (h*dh, 256) = vw^T @ E; y (q, d1) accumulated by 8 matmuls
    lhsT = O[:, h-block], rhs = Wo with all rows but head h's zeroed
    (host-built), which does the per-head diagonal extraction for free.
"""

import numpy as np

B, M, D1, D2 = 1, 2048, 128, 16
NQ, NK, H, DH = 32, 128, 8, 16
C = 64
NCORES = 8
W = C // NCORES  # windows per core
NCHUNK = 4       # q-chunks of 8 for the pair-bias matmuls
SCOL = H * NQ    # 256 score columns, layout h*32+q

_CACHE: dict = {}


def _build_nc():
    import contextlib
    import concourse.bass as bass
    import concourse.mybir as mybir
    import concourse.tile as tile
    from concourse.masks import make_identity

    f32 = mybir.dt.float32
    bf16 = mybir.dt.bfloat16
    A = mybir.AluOpType
    AF = mybir.ActivationFunctionType

    nc = bass.Bass()
    qlw = nc.declare_dram_parameter("qlw", [W, NQ, D1], bf16, isOutput=False)
    clw = nc.declare_dram_parameter("clw", [W, NK, D1], bf16, isOutput=False)
    pwq = nc.declare_dram_parameter("pwq", [W, 128, NCHUNK * NK], bf16, isOutput=False)
    kmask = nc.declare_dram_parameter("kmask", [NK, W], f32, isOutput=False)
    wq = nc.declare_dram_parameter("wq", [D1, H * DH], bf16, isOutput=False)
    wk = nc.declare_dram_parameter("wk", [D1, H * DH], bf16, isOutput=False)
    wv = nc.declare_dram_parameter("wv", [D1, H * DH], bf16, isOutput=False)
    wbd = nc.declare_dram_parameter("wbd", [NCHUNK, 128, SCOL], bf16, isOutput=False)
    qmsk = nc.declare_dram_parameter("qmsk", [128, SCOL], bf16, isOutput=False)
    wom = nc.declare_dram_parameter("wom", [H, H * DH, D1], bf16, isOutput=False)
    y = nc.declare_dram_parameter("y", [W, NQ, D1], f32, isOutput=True)

    with tile.TileContext(nc) as tc:
        with contextlib.ExitStack() as ctx:
            consts = ctx.enter_context(tc.tile_pool(name="consts", bufs=1))
            io = ctx.enter_context(tc.tile_pool(name="io", bufs=3))
            work = ctx.enter_context(tc.tile_pool(name="work", bufs=2))
            ps = ctx.enter_context(tc.tile_pool(name="ps", bufs=2, space="PSUM"))

            ident = consts.tile([128, 128], bf16)
            make_identity(nc, ident[:])
            wq_sb = consts.tile([128, 128], bf16)
            nc.sync.dma_start(out=wq_sb[:], in_=wq[:, :])
            wk_sb = consts.tile([128, 128], bf16)
            nc.sync.dma_start(out=wk_sb[:], in_=wk[:, :])
            wv_sb = consts.tile([128, 128], bf16)
            nc.sync.dma_start(out=wv_sb[:], in_=wv[:, :])
            wbd_sb = consts.tile([128, NCHUNK, SCOL], bf16)
            nc.sync.dma_start(
                out=wbd_sb[:],
                in_=bass.AP(
                    tensor=wbd[:, :, :].tensor,
                    offset=0,
                    ap=[[SCOL, 128], [128 * SCOL, NCHUNK], [1, SCOL]],
                ),
            )
            qm_sb = consts.tile([128, SCOL], bf16)
            nc.sync.dma_start(out=qm_sb[:], in_=qmsk[:, :])
            woM = []
            for j in range(H):
                t = consts.tile([128, 128], bf16, tag=f"woM{j}")
                nc.sync.dma_start(out=t[:], in_=wom[j, :, :])
                woM.append(t)
            kmt = consts.tile([NK, W], f32)
            nc.sync.dma_start(out=kmt[:], in_=kmask[:, :])
            ones_sb = consts.tile([128, 128], bf16)
            nc.vector.memset(ones_sb[:], 1.0)
            eps_t = consts.tile([128, 1], f32)
            nc.vector.memset(eps_t[:], 1e-5)

            for w in range(W):
                qlt = io.tile([NQ, D1], bf16)
                nc.sync.dma_start(out=qlt[:], in_=qlw[w, :, :])
                clt = io.tile([NK, D1], bf16)
                nc.sync.dma_start(out=clt[:], in_=clw[w, :, :])
                pwt = io.tile([128, NCHUNK * NK], bf16)
                nc.sync.dma_start(out=pwt[:], in_=pwq[w, :, :])

                # --- LayerNorm of the q rows and kv rows ---
                stq = work.tile([NQ, 6], f32)
                nc.vector.bn_stats(out=stq[:], in_=qlt[:])
                mvq = work.tile([NQ, 2], f32)
                nc.vector.bn_aggr(out=mvq[:], in_=stq[:])
                nc.scalar.activation(out=mvq[:, 1:2], in_=mvq[:, 1:2],
                                     func=AF.Sqrt, bias=eps_t[:NQ], scale=1.0)
                nc.vector.reciprocal(out=mvq[:, 1:2], in_=mvq[:, 1:2])
                lnq = work.tile([NQ, D1], bf16)
                nc.vector.tensor_scalar(out=lnq[:], in0=qlt[:],
                                        scalar1=mvq[:, 0:1], scalar2=mvq[:, 1:2],
                                        op0=A.subtract, op1=A.mult)

                stk = work.tile([NK, 6], f32)
                nc.vector.bn_stats(out=stk[:], in_=clt[:])
                mvk = work.tile([NK, 2], f32)
                nc.vector.bn_aggr(out=mvk[:], in_=stk[:])
                nc.scalar.activation(out=mvk[:, 1:2], in_=mvk[:, 1:2],
                                     func=AF.Sqrt, bias=eps_t[:NK], scale=1.0)
                nc.vector.reciprocal(out=mvk[:, 1:2], in_=mvk[:, 1:2])
                lnk = work.tile([NK, D1], bf16)
                nc.vector.tensor_scalar(out=lnk[:], in0=clt[:],
                                        scalar1=mvk[:, 0:1], scalar2=mvk[:, 1:2],
                                        op0=A.subtract, op1=A.mult)

                # --- transposes to (d1, rows) ---
                lnqT_ps = ps.tile([128, NQ], bf16, tag="tp")
                nc.tensor.transpose(out=lnqT_ps[:], in_=lnq[:],
                                    identity=ident[:NQ, :NQ])
                lnqT = work.tile([128, NQ], bf16)
                nc.vector.tensor_copy(out=lnqT[:], in_=lnqT_ps[:])

                lnkT_ps = ps.tile([128, NK], bf16, tag="tp")
                nc.tensor.transpose(out=lnkT_ps[:], in_=lnk[:],
                                    identity=ident[:NK, :NK])
                lnkT = work.tile([128, NK], bf16)
                nc.vector.tensor_copy(out=lnkT[:], in_=lnkT_ps[:])

                # --- projections ---
                qwT_ps = ps.tile([128, NQ], f32, tag="proj")
                nc.tensor.matmul(qwT_ps[:], lhsT=wq_sb[:], rhs=lnqT[:],
                                 start=True, stop=True)
                qwT = work.tile([128, NQ], bf16)
                nc.scalar.copy(out=qwT[:], in_=qwT_ps[:])

                kwT_ps = ps.tile([128, NK], f32, tag="proj")
                nc.tensor.matmul(kwT_ps[:], lhsT=wk_sb[:], rhs=lnkT[:],
                                 start=True, stop=True)
                kwT = work.tile([128, NK], bf16)
                nc.scalar.copy(out=kwT[:], in_=kwT_ps[:])

                vw_ps = ps.tile([NK, 128], f32, tag="proj")
                nc.tensor.matmul(vw_ps[:], lhsT=lnkT[:], rhs=wv_sb[:],
                                 start=True, stop=True)
                vw = work.tile([NK, 128], bf16)
                nc.scalar.copy(out=vw[:], in_=vw_ps[:])

                # --- block-diagonal q operand, one masked broadcast multiply
                qbd = work.tile([128, SCOL], bf16)
                mv = qm_sb[:]
                mv = bass.AP(tensor=mv.tensor, offset=mv.offset,
                             ap=[mv.ap[0], [NQ, H], [1, NQ]])
                ov = qbd[:]
                ov = bass.AP(tensor=ov.tensor, offset=ov.offset,
                             ap=[ov.ap[0], [NQ, H], [1, NQ]])
                qv = qwT[:]
                qv = bass.AP(tensor=qv.tensor, offset=qv.offset,
                             ap=[qv.ap[0], [0, H], [1, NQ]])
                nc.vector.tensor_tensor(out=ov, in0=mv, in1=qv, op=A.mult)

                # --- scores + pair bias, single PSUM accumulation chain ---
                S_ps = ps.tile([NK, SCOL], f32, tag="att")
                for i in range(NCHUNK):
                    nc.tensor.matmul(S_ps[:],
                                     lhsT=pwt[:, i * NK:(i + 1) * NK],
                                     rhs=wbd_sb[:, i, :],
                                     start=(i == 0), stop=False)
                nc.tensor.matmul(S_ps[:], lhsT=kwT[:], rhs=qbd[:],
                                 start=False, stop=True)

                # --- exp, then zero masked-key rows (the reference alpha_mask
                # factorizes as query_mask x key_mask; the query side is
                # applied on the host after the scatter) ---
                E_sb = work.tile([NK, SCOL], bf16)
                nc.scalar.activation(out=E_sb[:], in_=S_ps[:], func=AF.Exp)
                nc.vector.tensor_scalar_mul(out=E_sb[:], in0=E_sb[:],
                                            scalar1=kmt[:, w:w + 1])

                # --- softmax denominator over k (partition axis) ---
                d_ps = ps.tile([NK, SCOL], f32, tag="small")
                nc.tensor.matmul(d_ps[:], lhsT=ones_sb[:], rhs=E_sb[:],
                                 start=True, stop=True)
                rd = work.tile([NK, SCOL], f32)
                nc.vector.reciprocal(out=rd[:], in_=d_ps[:])
                nc.vector.tensor_mul(out=E_sb[:], in0=E_sb[:], in1=rd[:])

                # --- PV ---
                O_ps = ps.tile([128, SCOL], f32, tag="att")
                nc.tensor.matmul(O_ps[:], lhsT=vw[:], rhs=E_sb[:],
                                 start=True, stop=True)
                O_sb = work.tile([128, SCOL], bf16)
                nc.scalar.copy(out=O_sb[:], in_=O_ps[:])

                # --- output projection; per-head zeroed Wo rows extract the
                # diagonal head blocks during the accumulation ---
                Y_ps = ps.tile([NQ, D1], f32, tag="small")
                for h in range(H):
                    nc.tensor.matmul(Y_ps[:],
                                     lhsT=O_sb[:, h * NQ:(h + 1) * NQ],
                                     rhs=woM[h][:],
                                     start=(h == 0), stop=(h == H - 1))
                Y_sb = io.tile([NQ, D1], f32)
                nc.scalar.copy(out=Y_sb[:], in_=Y_ps[:])
                nc.sync.dma_start(out=y[w, :, :], in_=Y_sb[:])

    return nc


def _split_multi_waits(nc, mybir):
    """Walrus's per-instruction sync encoding rejects more than one wait
    condition; spill surplus waits onto same-engine NoOps placed immediately
    before (identical gating, the sequencer processes them in order)."""
    skip = ("InstNoOp", "InstCall")
    ctr = 0
    for b in nc.m.functions[0].blocks:
        insts = b.instructions
        for idx in range(len(insts) - 1, -1, -1):
            inst = insts[idx]
            si = inst.sync_info
            if si is None or len(si.on_wait) <= 1:
                continue
            if type(inst).__name__ in skip:
                continue
            waits = list(si.on_wait)
            inst.sync_info = mybir.SyncInfo(on_wait=[waits[-1]],
                                            on_update=list(si.on_update))
            for wcond in waits[:-1]:
                ctr += 1
                nop = mybir.InstNoOp(
                    name=f"waitsplit-{ctr}", engine=inst.engine, ins=[], outs=[],
                    bass_nofuse=True,
                    sync_info=mybir.SyncInfo(on_wait=[wcond], on_update=[]))
                insts.insert(idx, nop)


def _get_nc():
    if "nc" not in _CACHE:
        import concourse.mybir as mybir
        nc = _build_nc()
        _split_multi_waits(nc, mybir)
        _CACHE["nc"] = nc
    return _CACHE["nc"]


def _host_prep(ql, cl, plm, Wq, Wk, Wv, Wb, Wo,
               query_idx, key_idx, alpha_mask, pair_idx):
    """Build the 8 per-core input maps (the shard step)."""
    import ml_dtypes
    bf = ml_dtypes.bfloat16
    f32 = np.float32

    ql0 = np.ascontiguousarray(np.asarray(ql, f32)[0])
    cl0 = np.ascontiguousarray(np.asarray(cl, f32)[0])
    plm_flat = np.asarray(plm, f32).reshape(M * M, D2)
    qi = np.asarray(query_idx).astype(np.int64)
    ki = np.asarray(key_idx).astype(np.int64)
    am = np.asarray(alpha_mask).astype(np.int64)
    pi = np.asarray(pair_idx).astype(np.int64)

    wq_b = (np.asarray(Wq, f32) * np.float32(1.0 / np.sqrt(DH))).astype(bf)
    wk_b = np.asarray(Wk, f32).astype(bf)
    wv_b = np.asarray(Wv, f32).astype(bf)
    Wb32 = np.asarray(Wb, f32)
    Wo32 = np.asarray(Wo, f32)

    # sparse block-diagonal Wb: wbd[i, q%8*16+d, h*32+q] = Wb[d, h]
    wbd = np.zeros((NCHUNK, 128, SCOL), f32)
    for i in range(NCHUNK):
        for qq in range(8):
            q = 8 * i + qq
            for h in range(H):
                wbd[i, qq * D2:(qq + 1) * D2, h * NQ + q] = Wb32[:, h]
    wbd = wbd.astype(bf)

    # 0/1 mask for the block-diagonal q operand
    qmsk = np.zeros((128, SCOL), f32)
    for h in range(H):
        qmsk[h * DH:(h + 1) * DH, h * NQ:(h + 1) * NQ] = 1
    qmsk = qmsk.astype(bf)

    # Wo with all rows but head h's zeroed
    hm = np.zeros((H, 128, 1), f32)
    for h in range(H):
        hm[h, h * DH:(h + 1) * DH] = 1
    wom = (Wo32[None, :, :] * hm).astype(bf)

    in_maps = []
    for s in range(NCORES):
        sl = slice(s * W, (s + 1) * W)
        qis, kis, pis, ams = qi[sl], ki[sl], pi[sl], am[sl]
        qlw = ql0[qis.reshape(-1)].reshape(W, NQ, D1).astype(bf)
        clw = cl0[kis.reshape(-1)].reshape(W, NK, D1).astype(bf)
        pw = plm_flat[pis.reshape(-1)].reshape(W, NQ, NK, D2)
        # (w, q, k, d) -> partition (q%8)*16+d, free (q//8)*128+k
        pwq = np.ascontiguousarray(
            pw.reshape(W, NCHUNK, 8, NK, D2).transpose(0, 2, 4, 1, 3)
        ).reshape(W, 128, NCHUNK * NK).astype(bf)
        km = (ams.max(axis=1) > 0).astype(f32)   # (W, NK) effective key mask
        in_maps.append({
            "qlw": qlw, "clw": clw, "pwq": pwq,
            "kmask": np.ascontiguousarray(km.T),
            "wq": wq_b, "wk": wk_b, "wv": wv_b, "wbd": wbd,
            "qmsk": qmsk, "wom": wom,
        })
    return in_maps


def _host_finish(y_all, ql, query_idx, query_mask):
    """Unshard: mask, scatter-add the query tiles, residual."""
    f32 = np.float32
    ql = np.asarray(ql, f32)
    qi = np.asarray(query_idx).astype(np.int64)
    qm = np.asarray(query_mask).astype(f32)
    o = np.zeros((M, D1), f32)
    yw = y_all.reshape(C * NQ, D1) * qm.reshape(C * NQ, 1)
    np.add.at(o, qi.reshape(-1), yw)
    return (ql + o[None]).astype(f32)


def kernel(ql, cl, plm, Wq, Wk, Wv, Wb, Wo,
           query_idx, query_mask, key_idx, key_mask, alpha_mask, pair_idx,
           **_unused):
    from concourse.bass_utils import run_bass_kernel_spmd

    in_maps = _host_prep(ql, cl, plm, Wq, Wk, Wv, Wb, Wo,
                         query_idx, key_idx, alpha_mask, pair_idx)
    nc = _get_nc()
    res = run_bass_kernel_spmd(nc, in_maps, list(range(NCORES)))
    y_all = np.concatenate(
        [np.asarray(res.results[s]["y"], np.float32) for s in range(NCORES)], axis=0
    )
    return _host_finish(y_all, ql, query_idx, query_mask)
